# revision 3
# baseline (speedup 1.0000x reference)
"""AngularMarginLoss (ArcFace-style) distributed Trainium2 kernel.

Strategy (class-parallel over 8 NeuronCores):
  - Host: L2-normalize x rows and W rows, transpose to [E, *] layouts,
    shard W columns (classes) across 8 cores.
  - Device (per core, C/8 = 12500 classes):
      cos chunk = xnT.T @ WnT_shard  (fp32r matmul, PSUM [128, 500])
      ScalarE: exp(30*cos) with accum_out -> per-chunk partial sums (free)
      VectorE: per-chunk top-8 of exp values -> chunk maxima
  - Host epilogue: combine partial sums; resolve argmax within candidate
    chunks (margin covers fp32r error); compute margin numerator and loss.

Self-contained: shapes hardcoded; device work runs in a subprocess so the
harness's jax state/env cannot interfere.
"""
import os
import subprocess
import sys
import tempfile

import numpy as np

B, E, C = 1024, 256, 100000
NCORES = 8
CS = C // NCORES            # 12500 classes per core
NCCH = 500                  # classes per chunk (1 PSUM bank)
NCHUNKS = CS // NCCH        # 25
NTILES = B // 128           # 8 B-tiles
NPS = 4                     # rotating PSUM banks / exp buffers
SCALE = 30.0
MARGIN = 0.5
EPS = 1e-6
DELTA_COS = 1e-3            # fp32r error margin for argmax chunk candidates


def _build_graph():
    import concourse.bass as bass
    import concourse.mybir as mybir

    nc = bass.Bass(target_bir_lowering=False)
    xt_ext = nc.declare_dram_parameter("xt", [E, B], mybir.dt.float32r, isOutput=False)
    wt_ext = nc.declare_dram_parameter("wt", [E, CS], mybir.dt.float32r, isOutput=False)
    o_acc = nc.declare_dram_parameter("o_acc", [128, NTILES * NCHUNKS], mybir.dt.float32, isOutput=True)
    o_max = nc.declare_dram_parameter("o_max", [128, NTILES * NCHUNKS * 8], mybir.dt.float32, isOutput=True)

    NITER = NCHUNKS * NTILES  # 200

    from contextlib import ExitStack
    with ExitStack() as ctx:
        xt0 = ctx.enter_context(nc.sbuf_tensor("xt0", [128, B], mybir.dt.float32r))
        xt1 = ctx.enter_context(nc.sbuf_tensor("xt1", [128, B], mybir.dt.float32r))
        wt0 = ctx.enter_context(nc.sbuf_tensor("wt0", [128, CS], mybir.dt.float32r))
        wt1 = ctx.enter_context(nc.sbuf_tensor("wt1", [128, CS], mybir.dt.float32r))
        accv = ctx.enter_context(nc.sbuf_tensor("accv", [128, NTILES * NCHUNKS], mybir.dt.float32))
        maxb = ctx.enter_context(nc.sbuf_tensor("maxb", [128, NTILES * NCHUNKS * 8], mybir.dt.float32))
        expb = [
            ctx.enter_context(nc.sbuf_tensor(f"expb{k}", [128, NCCH], mybir.dt.float32))
            for k in range(NPS)
        ]
        ps = [
            ctx.enter_context(nc.psum_tensor(f"ps{k}", [128, NCCH], mybir.dt.float32))
            for k in range(NPS)
        ]
        dma_sem = ctx.enter_context(nc.semaphore("dma_sem"))
        mm_sem = ctx.enter_context(nc.semaphore("mm_sem"))
        act_sem = ctx.enter_context(nc.semaphore("act_sem"))
        dve_sem = ctx.enter_context(nc.semaphore("dve_sem"))
        block = ctx.enter_context(nc.Block())

        @block.sync
        def _(sync):
            sync.dma_start(out=xt0[:], in_=xt_ext[0:128, :]).then_inc(dma_sem, 16)
            sync.dma_start(out=xt1[:], in_=xt_ext[128:256, :]).then_inc(dma_sem, 16)
            for c in range(NCHUNKS):
                sl = slice(c * NCCH, (c + 1) * NCCH)
                sync.dma_start(out=wt0[:, sl], in_=wt_ext[0:128, sl]).then_inc(dma_sem, 16)
                sync.dma_start(out=wt1[:, sl], in_=wt_ext[128:256, sl]).then_inc(dma_sem, 16)
            sync.wait_ge(act_sem, NITER)
            sync.dma_start(out=o_acc[:], in_=accv[:]).then_inc(dma_sem, 16)
            sync.wait_ge(dve_sem, NITER)
            sync.dma_start(out=o_max[:], in_=maxb[:]).then_inc(dma_sem, 16)

        @block.tensor
        def _(tensor):
            i = 0
            for c in range(NCHUNKS):
                tensor.wait_ge(dma_sem, 16 * (2 + 2 * (c + 1)))
                sl = slice(c * NCCH, (c + 1) * NCCH)
                for t in range(NTILES):
                    if i >= NPS:
                        tensor.wait_ge(act_sem, i - NPS + 1)
                    p = ps[i % NPS]
                    tsl = slice(t * 128, (t + 1) * 128)
                    tensor.matmul(p[:], xt0[:, tsl], wt0[:, sl], start=True, stop=False)
                    tensor.matmul(p[:], xt1[:, tsl], wt1[:, sl], start=False, stop=True).then_inc(mm_sem, 1)
                    i += 1

        @block.scalar
        def _(scalar):
            import concourse.mybir as mybir
            i = 0
            for c in range(NCHUNKS):
                for t in range(NTILES):
                    scalar.wait_ge(mm_sem, i + 1)
                    if i >= NPS:
                        scalar.wait_ge(dve_sem, i - NPS + 1)
                    col = t * NCHUNKS + c
                    scalar.activation(
                        expb[i % NPS][:], ps[i % NPS][:],
                        mybir.ActivationFunctionType.Exp,
                        scale=SCALE, accum_out=accv[:, col:col + 1],
                    ).then_inc(act_sem, 1)
                    i += 1

        @block.vector
        def _(vector):
            i = 0
            for c in range(NCHUNKS):
                for t in range(NTILES):
                    vector.wait_ge(act_sem, i + 1)
                    col = t * NCHUNKS + c
                    vector.max(maxb[:, col * 8:(col + 1) * 8], expb[i % NPS][:]).then_inc(dve_sem, 1)
                    i += 1

    return nc


def _worker(tmpdir):
    trace = os.environ.get("AML_TRACE", "0") == "1"
    if trace:
        # Dev-only: wire up the NTFF profile hook that this image's antenv
        # lacks, and stub the artifact upload (no bucket access here).
        try:
            import types

            import trn_agent_boot.trn_boot as tb

            hook = tb._ntff_profile_via_ctypes("/opt/axon/libaxon_pjrt.so")
            mod = types.ModuleType("antenv.axon_hooks")
            mod.get_axon_ntff_profile_hook = lambda: hook
            sys.modules["antenv.axon_hooks"] = mod
            import concourse.bass_utils as _bu

            _bu.upload_artifacts = lambda d: "local://" + d
        except Exception as e:  # pragma: no cover
            print("trace setup failed, disabling trace:", e)
            trace = False

    from concourse.bass_utils import run_bass_kernel_spmd

    xnT = np.load(os.path.join(tmpdir, "xnT.npy"))
    WnT = np.load(os.path.join(tmpdir, "WnT.npy"))
    nc = _build_graph()
    in_maps = [
        {"xt": xnT, "wt": np.ascontiguousarray(WnT[:, k * CS:(k + 1) * CS])}
        for k in range(NCORES)
    ]
    try:
        res = run_bass_kernel_spmd(nc, in_maps, core_ids=list(range(NCORES)), trace=trace)
    except Exception:
        if not trace:
            raise
        print("trace run failed; retrying without trace")
        res = run_bass_kernel_spmd(nc, in_maps, core_ids=list(range(NCORES)), trace=False)
    acc = np.stack([r["o_acc"] for r in res.results])   # [8, 128, 200]
    mx = np.stack([r["o_max"] for r in res.results])    # [8, 128, 1600]
    np.save(os.path.join(tmpdir, "acc.npy"), acc)
    np.save(os.path.join(tmpdir, "max.npy"), mx)
    if res.exec_time_ns is not None:
        with open(os.path.join(tmpdir, "exec_time_ns.txt"), "w") as f:
            f.write(str(res.exec_time_ns))


def kernel(inputs, targets, W):
    x = np.asarray(inputs)
    tg = np.asarray(targets).astype(np.int64)
    Wf = np.asarray(W)

    x64 = x.astype(np.float64)
    W64 = Wf.astype(np.float64)
    xn64 = x64 / np.linalg.norm(x64, axis=1, keepdims=True)
    Wn64 = W64 / np.linalg.norm(W64, axis=1, keepdims=True)
    xnT = np.ascontiguousarray(xn64.T.astype(np.float32))   # [E, B]
    WnT = np.ascontiguousarray(Wn64.T.astype(np.float32))   # [E, C]

    tmpdir = tempfile.mkdtemp(prefix="aml_")
    np.save(os.path.join(tmpdir, "xnT.npy"), xnT)
    np.save(os.path.join(tmpdir, "WnT.npy"), WnT)
    env = dict(os.environ)
    env["JAX_PLATFORMS"] = "axon"
    subprocess.run(
        [sys.executable, os.path.abspath(__file__), "--worker", tmpdir],
        check=True, env=env,
    )
    acc = np.load(os.path.join(tmpdir, "acc.npy"))   # [8, 128, 200] col=t*25+c
    mx = np.load(os.path.join(tmpdir, "max.npy"))    # [8, 128, 1600]

    # --- partial sums -> full sum_exp per sample (sample i = t*128 + p) ---
    acc4 = acc.reshape(NCORES, 128, NTILES, NCHUNKS).astype(np.float64)
    sums_cpt = acc4.sum(axis=3)                       # [core, p, t]
    S = sums_cpt.sum(axis=0).T.reshape(B)             # [p,t] -> [t,p] -> flat i=t*128+p

    # chunk maxima: top1 of each chunk
    mx5 = mx.reshape(NCORES, 128, NTILES, NCHUNKS, 8)[..., 0]  # [core, p, t, c]
    # arrange as [sample, core, c]
    cmax = np.transpose(mx5, (2, 1, 0, 3)).reshape(B, NCORES, NCHUNKS)

    # --- argmax via candidate chunks + exact rescore ---
    cand_flat = cmax.reshape(B, NCORES * NCHUNKS)     # chunk id g = core*25 + c
    gmax = cand_flat.max(axis=1)
    thresh = gmax * np.exp(-SCALE * DELTA_COS)
    preds = np.full(B, -1, dtype=np.int64)
    best = np.full(B, -np.inf)
    cand_mask = cand_flat >= thresh[:, None]
    # iterate over unique candidate chunks, ascending global order
    for g in np.nonzero(cand_mask.any(axis=0))[0]:
        rows = np.nonzero(cand_mask[:, g])[0]
        core, c = divmod(g, NCHUNKS)
        base = core * CS + c * NCCH
        blockW = Wn64[base:base + NCCH]               # [500, 256]
        scores = xn64[rows] @ blockW.T                # [nrows, 500]
        loc = np.argmax(scores, axis=1)
        val = scores[np.arange(len(rows)), loc]
        upd = val > best[rows]
        ridx = rows[upd]
        best[ridx] = val[upd]
        preds[ridx] = base + loc[upd]

    # --- loss ---
    cos_t = np.einsum("ij,ij->i", xn64, Wn64[tg])
    cos_t = np.clip(cos_t, -1.0, 1.0)
    num = SCALE * (np.cos(np.arccos(cos_t) + MARGIN))
    sum_excl = S - np.exp(SCALE * cos_t)
    den = np.exp(num) + sum_excl
    loss = -np.mean(num - np.log(den + EPS))

    return inputs, preds.astype(np.int32), np.float32(loss)


if __name__ == "__main__":
    if len(sys.argv) >= 3 and sys.argv[1] == "--worker":
        _worker(sys.argv[2])
